# revision 13
# baseline (speedup 1.0000x reference)
"""Trainium2 Bass kernel for nn_MetaHeteroLinear (moe_routing).

out[n] = x[n] @ W[type_vec[n]] + B[type_vec[n]],
with W [8,128,128] / B [8,128] generated from edge_feas by two small MLPs.

Strategy (8 NeuronCores, data parallel over rows; 62500 rows/core):
 - The generator MLPs are tiny (~70 MFLOP total); computed once on host in
   f32 and the resulting per-type W/B replicated to every core (per the
   sharding hint) — this avoids shipping the 16 MB wg_w3 weight 8x per call.
 - Host computes routing tables (argsort by type per half-shard so gather
   indices fit int16) and per-call valid counts.
 - Device, per (half-shard, type) call: transposed dma_gather pulls the
   rows of that type as x^T columns (bf16), 33 matmul tiles of 128 rows
   against the resident W[t] with the bias folded in via a 1-row seed
   matmul into PSUM, then dma_scatter_add writes each row's result
   directly to its natural output position (the output buffer is donated
   zero-filled, so += on untouched rows == store). Padding tokens are -1
   (skipped by both gather and scatter); per-call valid counts are loaded
   into gpsimd registers at runtime.
 - Everything moves as bf16 (rel-err ~3e-3, well under the 2e-2 gate),
   halving both tunnel directions vs f32.
 - The jit-wrapped NEFF executable is cached across calls; output zeros
   are produced on-device (never shipped); output fetch is threaded.
"""
import numpy as np
import ml_dtypes

import jax
import jax.numpy as jnp
from jax.experimental.shard_map import shard_map
from jax.sharding import Mesh, PartitionSpec, NamedSharding

import concourse.bass as bass
import concourse.bacc as bacc
import concourse.tile as tile
import concourse.mybir as mybir
from concourse import bass2jax

P = 128
IN_C = 128
OUT_C = 128
MEM = 512
HID = 256
T = 8

N_CORES = 8
N = 500_000
R = N // N_CORES           # 62500 rows per core
SUB = R // 2               # 31250: half-shards so gather idx fits int16
TPT = 33                   # tiles (of 128 rows) per (half, type) call
CAP = TPT * P              # 4224 row capacity per call (mean 3906 + 5.4 sigma)
NCALLS = 2 * T             # 16 calls per core
COLS = CAP // 16           # 264 idx columns per call

f32 = mybir.dt.float32
bf16 = mybir.dt.bfloat16
i16 = mybir.dt.int16
i32 = mybir.dt.int32
BF16 = ml_dtypes.bfloat16

_CACHE = {}
LAST_RESULTS = None  # kept for test harness compat (no NTFF profile available)


def _build_nc():
    nc = bacc.Bacc("TRN2", target_bir_lowering=False, debug=False)
    x_d = nc.dram_tensor("x_s", [R, IN_C], bf16, kind="ExternalInput")
    g1_d = nc.dram_tensor("g1idx", [16, NCALLS * COLS], i16, kind="ExternalInput")
    cnt_d = nc.dram_tensor("cnt", [1, NCALLS], i32, kind="ExternalInput")
    w_d = nc.dram_tensor("wt", [IN_C, T, OUT_C], bf16, kind="ExternalInput")
    b_d = nc.dram_tensor("bt", [1, T * OUT_C], bf16, kind="ExternalInput")
    out_d = nc.dram_tensor("out_s", [R, OUT_C], bf16, kind="ExternalOutput")

    with tile.TileContext(nc) as tc:
        with tc.tile_pool(name="const", bufs=1) as cpool, \
             tc.tile_pool(name="io", bufs=3) as iopool, \
             tc.tile_pool(name="ps", bufs=4, space="PSUM") as pspool:
            g1_sb = cpool.tile([P, NCALLS * COLS], i16)
            for g in range(8):  # replicate idx rows to all 8 Q7 core groups
                nc.sync.dma_start(out=g1_sb[g * 16:(g + 1) * 16, :], in_=g1_d[:])
            cnt_sb = cpool.tile([1, NCALLS], i32)
            nc.sync.dma_start(out=cnt_sb[:], in_=cnt_d[:])
            wcat_sb = cpool.tile([P, T, OUT_C], bf16)   # [in_c, t, out_c]
            nc.sync.dma_start(out=wcat_sb[:], in_=w_d[:])
            bt_sb = cpool.tile([1, T * OUT_C], bf16)  # all biases on partition 0
            nc.sync.dma_start(out=bt_sb[:], in_=b_d[:])
            ones_sb = cpool.tile([1, P], bf16)
            nc.vector.memset(ones_sb[:], 1.0)

            regs = [nc.gpsimd.alloc_register(f"cnt{k}") for k in range(NCALLS)]
            pend = None  # (y_sb, idx slice, reg, out AP) awaiting scatter
            for call in range(NCALLS):
                sub, t = divmod(call, T)
                lo = sub * SUB
                hi = R if sub == 1 else SUB
                r = regs[call]
                nc.gpsimd.reg_load(r, cnt_sb[:1, call:call + 1])
                xT = iopool.tile([P, 1, CAP], bf16, tag="xT")
                idx_ap = g1_sb[:, call * COLS:(call + 1) * COLS]
                nc.gpsimd.dma_gather(
                    out_ap=xT[:], in_ap=x_d[lo:hi, :], idxs_ap=idx_ap,
                    num_idxs=CAP, num_idxs_reg=r, elem_size=IN_C,
                    transpose=True, single_packet=False)
                y_sb = iopool.tile([P, TPT, OUT_C], bf16, tag="y")
                for j in range(TPT):
                    ps = pspool.tile([P, OUT_C], f32, tag="ps")
                    nc.tensor.matmul(ps[:], lhsT=ones_sb[:1, :],
                                     rhs=bt_sb[:1, t * OUT_C:(t + 1) * OUT_C],
                                     start=True, stop=False)
                    nc.tensor.matmul(ps[:], lhsT=xT[:, 0, j * P:(j + 1) * P],
                                     rhs=wcat_sb[:, t, :], start=False, stop=True)
                    nc.scalar.copy(y_sb[:, j, :], ps[:])
                # issue the previous call's scatter after this call's gather so
                # the gather DMA overlaps the previous call's matmul tail
                if pend is not None:
                    nc.gpsimd.dma_scatter_add(
                        out_ap=pend[3], in_ap=pend[0][:], idxs_ap=pend[1],
                        num_idxs=CAP, num_idxs_reg=pend[2], elem_size=OUT_C,
                        single_packet=False)
                pend = (y_sb, idx_ap, r, out_d[lo:hi, :])
            nc.gpsimd.dma_scatter_add(
                out_ap=pend[3], in_ap=pend[0][:], idxs_ap=pend[1],
                num_idxs=CAP, num_idxs_reg=pend[2], elem_size=OUT_C,
                single_packet=False)
    nc.compile()
    return nc


def _make_runner():
    """Compile once; return (sharded_jit, zeros_fn, in_names)."""
    bass2jax.install_neuronx_cc_hook()
    nc = _build_nc()
    assert nc.dbg_addr is None
    part_name = nc.partition_id_tensor.name if nc.partition_id_tensor else None
    in_names, out_names, out_avals = [], [], []
    for alloc in nc.m.functions[0].allocations:
        if not isinstance(alloc, mybir.MemoryLocationSet):
            continue
        name = alloc.memorylocations[0].name
        if alloc.kind == "ExternalInput":
            if name != part_name:
                in_names.append(name)
        elif alloc.kind == "ExternalOutput":
            out_names.append(name)
            out_avals.append(jax.core.ShapedArray(
                tuple(alloc.tensor_shape), mybir.dt.np(alloc.dtype)))
    n_params, n_outs = len(in_names), len(out_names)
    all_names = in_names + out_names
    if part_name is not None:
        all_names = all_names + [part_name]
    all_names = tuple(all_names)

    def _body(*args):
        operands = list(args)
        if part_name is not None:
            operands.append(bass2jax.partition_id_tensor())
        return tuple(bass2jax._bass_exec_p.bind(
            *operands, out_avals=tuple(out_avals), in_names=all_names,
            out_names=tuple(out_names), lowering_input_output_aliases=(),
            sim_require_finite=True, sim_require_nnan=True, nc=nc))

    mesh = Mesh(np.asarray(jax.devices()[:N_CORES]), ("core",))
    spec = PartitionSpec("core")
    sharded = jax.jit(
        shard_map(_body, mesh=mesh, in_specs=(spec,) * (n_params + n_outs),
                  out_specs=(spec,) * n_outs, check_rep=False),
        donate_argnums=tuple(range(n_params, n_params + n_outs)),
        keep_unused=True)
    zeros_fn = jax.jit(lambda: jnp.zeros((N_CORES * R, OUT_C), jnp.bfloat16),
                       out_shardings=NamedSharding(mesh, spec))
    _CACHE["mesh_spec"] = (mesh, spec)
    return sharded, zeros_fn, in_names


def _routing(tv_core):
    """tv_core: [R] int types -> (g1 [NCALLS, CAP] i16 with -1 pads,
    cnt [NCALLS] i32, overflow core-local row ids needing host fixup)."""
    g1 = np.full((NCALLS, CAP), -1, np.int16)
    cnt = np.zeros(NCALLS, np.int32)
    overflow = []
    for sub in range(2):
        lo, hi = sub * SUB, (R if sub == 1 else SUB)
        tvs = tv_core[lo:hi]
        order = np.argsort(tvs, kind="stable")
        counts = np.bincount(tvs, minlength=T)
        start = 0
        for t in range(T):
            c = int(counts[t])
            seg = order[start:start + c]
            start += c
            k = sub * T + t
            if c > CAP:
                overflow.extend((seg[CAP:] + lo).tolist())
                seg, c = seg[:CAP], CAP
            if c == 0:
                # hardware path needs >=1 valid token per call; sacrifice
                # local row 0 (scatter adds garbage there; host recomputes)
                g1[k, 0] = 0
                cnt[k] = 1
                overflow.append(lo)
            else:
                g1[k, :c] = seg.astype(np.int16)
                cnt[k] = c
    return g1, cnt, overflow


def _wrap16(flat):
    """flat int16 [NCALLS*CAP] -> [16, NCALLS*COLS] wrapped (token i at
    [i%16, i//16]); replication to the 8 Q7 core groups happens on device."""
    return flat.reshape(-1, 16).T


def _host_mlp(m, w1, b1, w2, b2, w3, b3):
    h = np.maximum(m @ w1 + b1, 0)
    h = np.maximum(h @ w2 + b2, 0)
    return h @ w3 + b3


def kernel(**inputs):
    x = np.ascontiguousarray(np.asarray(inputs["x"], dtype=np.float32))
    tv = np.asarray(inputs["type_vec"]).astype(np.int64)
    assert x.shape == (N, IN_C), x.shape
    ef = np.asarray(inputs["edge_feas"], dtype=np.float32)

    # per-type weights/biases from the tiny generator MLPs (host, f32)
    W = _host_mlp(ef, *[np.asarray(inputs[k], dtype=np.float32) for k in
                        ("wg_w1", "wg_b1", "wg_w2", "wg_b2", "wg_w3", "wg_b3")]
                  ).reshape(T, IN_C, OUT_C)
    B = _host_mlp(ef, *[np.asarray(inputs[k], dtype=np.float32) for k in
                        ("bg_w1", "bg_b1", "bg_w2", "bg_b2", "bg_w3", "bg_b3")])

    if "runner" not in _CACHE:
        _CACHE["runner"] = _make_runner()
    sharded, zeros_fn, in_names = _CACHE["runner"]

    zeros = zeros_fn()  # async on-device; overlaps with host prep below
    # enqueue the big x transfer before doing routing work on host
    mesh, spec = _CACHE["mesh_spec"]
    x_dev = jax.device_put(x.astype(BF16), NamedSharding(mesh, spec))

    g1_g = np.empty((N_CORES * 16, NCALLS * COLS), np.int16)
    cnt_g = np.empty((N_CORES, NCALLS), np.int32)
    overflows = []
    for c in range(N_CORES):
        g1, cnt, ovf = _routing(tv[c * R:(c + 1) * R])
        g1_g[c * 16:(c + 1) * 16] = _wrap16(g1.reshape(-1))
        cnt_g[c] = cnt
        overflows.append(ovf)

    w_g = np.broadcast_to(
        np.ascontiguousarray(W.transpose(1, 0, 2)).astype(BF16),
        (N_CORES, IN_C, T, OUT_C)).reshape(N_CORES * IN_C, T, OUT_C)
    b_g = np.broadcast_to(B.reshape(1, T * OUT_C).astype(BF16),
                          (N_CORES, T * OUT_C))

    glob = {"x_s": x_dev, "g1idx": g1_g, "cnt": cnt_g,
            "wt": np.ascontiguousarray(w_g), "bt": np.ascontiguousarray(b_g)}
    out_arr = sharded(*[glob[n] for n in in_names], zeros)[0]

    out = np.empty((N, OUT_C), dtype=np.float32)
    from concurrent.futures import ThreadPoolExecutor

    def fetch(s):
        lo = s.index[0].start or 0
        out[lo:lo + R] = np.asarray(s.data)  # bf16 -> f32 cast on assign

    with ThreadPoolExecutor(4) as ex:
        list(ex.map(fetch, out_arr.addressable_shards))

    if any(overflows):  # per-type capacity overflow: recompute those rows
        for c in range(N_CORES):
            for rr in set(overflows[c]):
                g = c * R + rr
                t = int(tv[g])
                out[g] = x[g] @ W[t] + B[t]
    return out


# revision 15
# speedup vs baseline: 1.2851x; 1.2851x over previous
"""Trainium2 Bass kernel for nn_MetaHeteroLinear (moe_routing).

out[n] = x[n] @ W[type_vec[n]] + B[type_vec[n]],
with W [8,128,128] / B [8,128] generated from edge_feas by two small MLPs.

Strategy (8 NeuronCores, data parallel over rows; 62500 rows/core):
 - The generator MLPs are tiny (~70 MFLOP total); computed once on host in
   f32 and the resulting per-type W/B replicated to every core (per the
   sharding hint) — this avoids shipping the 16 MB wg_w3 weight 8x per call.
 - Host computes routing tables (argsort by type per half-shard so gather
   indices fit int16) and per-call valid counts.
 - Device, per (half-shard, type) call: transposed dma_gather pulls the
   rows of that type as x^T columns (bf16), 33 matmul tiles of 128 rows
   against the resident W[t] with the bias folded in via a 1-row seed
   matmul into PSUM, then dma_scatter_add writes each row's result
   directly to its natural output position (the output buffer is donated
   zero-filled, so += on untouched rows == store). Padding tokens are -1
   (skipped by both gather and scatter); per-call valid counts are loaded
   into gpsimd registers at runtime.
 - Everything moves as bf16 (rel-err ~3e-3, well under the 2e-2 gate),
   halving both tunnel directions vs f32.
 - The jit-wrapped NEFF executable is cached across calls; output zeros
   are produced on-device (never shipped); output fetch is threaded.
"""
import numpy as np
import ml_dtypes

import jax
import jax.numpy as jnp
from jax.experimental.shard_map import shard_map
from jax.sharding import Mesh, PartitionSpec, NamedSharding

import concourse.bacc as bacc
import concourse.tile as tile
import concourse.mybir as mybir
from concourse import bass2jax

P = 128
IN_C = 128
OUT_C = 128
MEM = 512
HID = 256
T = 8

N_CORES = 8
N = 500_000
R = N // N_CORES           # 62500 rows per core
SUB = R // 2               # 31250: half-shards so gather idx fits int16
TPT = 33                   # tiles (of 128 rows) per (half, type) call
CAP = TPT * P              # 4224 row capacity per call (mean 3906 + 5.4 sigma)
NCALLS = 2 * T             # 16 calls per core
COLS = CAP // 16           # 264 idx columns per call

f32 = mybir.dt.float32
bf16 = mybir.dt.bfloat16
i16 = mybir.dt.int16
i32 = mybir.dt.int32
BF16 = ml_dtypes.bfloat16

_CACHE = {}
LAST_RESULTS = None  # kept for test harness compat (no NTFF profile available)


def _build_nc():
    nc = bacc.Bacc("TRN2", target_bir_lowering=False, debug=False)
    x_d = nc.dram_tensor("x_s", [R, IN_C], bf16, kind="ExternalInput")
    g1_d = nc.dram_tensor("g1idx", [16, NCALLS * COLS], i16, kind="ExternalInput")
    cnt_d = nc.dram_tensor("cnt", [1, NCALLS], i32, kind="ExternalInput")
    w_d = nc.dram_tensor("wt", [IN_C, T, OUT_C], bf16, kind="ExternalInput")
    b_d = nc.dram_tensor("bt", [1, T * OUT_C], bf16, kind="ExternalInput")
    out_d = nc.dram_tensor("out_s", [R, OUT_C], bf16, kind="ExternalOutput")

    with tile.TileContext(nc) as tc:
        with tc.tile_pool(name="const", bufs=1) as cpool, \
             tc.tile_pool(name="io", bufs=3) as iopool, \
             tc.tile_pool(name="ps", bufs=4, space="PSUM") as pspool:
            g1_sb = cpool.tile([P, NCALLS * COLS], i16)
            for g in range(8):  # replicate idx rows to all 8 Q7 core groups
                nc.sync.dma_start(out=g1_sb[g * 16:(g + 1) * 16, :], in_=g1_d[:])
            cnt_sb = cpool.tile([1, NCALLS], i32)
            nc.sync.dma_start(out=cnt_sb[:], in_=cnt_d[:])
            wcat_sb = cpool.tile([P, T, OUT_C], bf16)   # [in_c, t, out_c]
            nc.sync.dma_start(out=wcat_sb[:], in_=w_d[:])
            bt_sb = cpool.tile([1, T * OUT_C], bf16)  # all biases on partition 0
            nc.sync.dma_start(out=bt_sb[:], in_=b_d[:])
            ones_sb = cpool.tile([1, P], bf16)
            nc.vector.memset(ones_sb[:], 1.0)

            regs = [nc.gpsimd.alloc_register(f"cnt{k}") for k in range(NCALLS)]
            pend = None  # (y_sb, idx slice, reg, out AP) awaiting scatter
            for call in range(NCALLS):
                sub, t = divmod(call, T)
                lo = sub * SUB
                hi = R if sub == 1 else SUB
                r = regs[call]
                nc.gpsimd.reg_load(r, cnt_sb[:1, call:call + 1])
                xT = iopool.tile([P, 1, CAP], bf16, tag="xT")
                idx_ap = g1_sb[:, call * COLS:(call + 1) * COLS]
                nc.gpsimd.dma_gather(
                    out_ap=xT[:], in_ap=x_d[lo:hi, :], idxs_ap=idx_ap,
                    num_idxs=CAP, num_idxs_reg=r, elem_size=IN_C,
                    transpose=True, single_packet=False)
                y_sb = iopool.tile([P, TPT, OUT_C], bf16, tag="y")
                for j in range(TPT):
                    ps = pspool.tile([P, OUT_C], f32, tag="ps")
                    nc.tensor.matmul(ps[:], lhsT=ones_sb[:1, :],
                                     rhs=bt_sb[:1, t * OUT_C:(t + 1) * OUT_C],
                                     start=True, stop=False)
                    nc.tensor.matmul(ps[:], lhsT=xT[:, 0, j * P:(j + 1) * P],
                                     rhs=wcat_sb[:, t, :], start=False, stop=True)
                    nc.scalar.copy(y_sb[:, j, :], ps[:])
                # issue the previous call's scatter after this call's gather so
                # the gather DMA overlaps the previous call's matmul tail
                if pend is not None:
                    nc.gpsimd.dma_scatter_add(
                        out_ap=pend[3], in_ap=pend[0][:], idxs_ap=pend[1],
                        num_idxs=CAP, num_idxs_reg=pend[2], elem_size=OUT_C,
                        single_packet=False)
                pend = (y_sb, idx_ap, r, out_d[lo:hi, :])
            nc.gpsimd.dma_scatter_add(
                out_ap=pend[3], in_ap=pend[0][:], idxs_ap=pend[1],
                num_idxs=CAP, num_idxs_reg=pend[2], elem_size=OUT_C,
                single_packet=False)
    nc.compile()
    return nc


def _make_runner():
    """Compile once; return (sharded_jit, zeros_fn, in_names)."""
    bass2jax.install_neuronx_cc_hook()
    nc = _build_nc()
    assert nc.dbg_addr is None
    part_name = nc.partition_id_tensor.name if nc.partition_id_tensor else None
    in_names, out_names, out_avals = [], [], []
    for alloc in nc.m.functions[0].allocations:
        if not isinstance(alloc, mybir.MemoryLocationSet):
            continue
        name = alloc.memorylocations[0].name
        if alloc.kind == "ExternalInput":
            if name != part_name:
                in_names.append(name)
        elif alloc.kind == "ExternalOutput":
            out_names.append(name)
            out_avals.append(jax.core.ShapedArray(
                tuple(alloc.tensor_shape), mybir.dt.np(alloc.dtype)))
    n_params, n_outs = len(in_names), len(out_names)
    all_names = in_names + out_names
    if part_name is not None:
        all_names = all_names + [part_name]
    all_names = tuple(all_names)

    def _body(*args):
        operands = list(args)
        if part_name is not None:
            operands.append(bass2jax.partition_id_tensor())
        return tuple(bass2jax._bass_exec_p.bind(
            *operands, out_avals=tuple(out_avals), in_names=all_names,
            out_names=tuple(out_names), lowering_input_output_aliases=(),
            sim_require_finite=True, sim_require_nnan=True, nc=nc))

    try:
        devs = jax.devices("neuron")
    except RuntimeError:
        devs = jax.devices()
    mesh = Mesh(np.asarray(devs[:N_CORES]), ("core",))
    spec = PartitionSpec("core")
    sharded = jax.jit(
        shard_map(_body, mesh=mesh, in_specs=(spec,) * (n_params + n_outs),
                  out_specs=(spec,) * n_outs, check_rep=False),
        donate_argnums=tuple(range(n_params, n_params + n_outs)),
        keep_unused=True)
    zeros_fn = jax.jit(lambda: jnp.zeros((N_CORES * R, OUT_C), jnp.bfloat16),
                       out_shardings=NamedSharding(mesh, spec))
    _CACHE["mesh_spec"] = (mesh, spec)
    return sharded, zeros_fn, in_names


def _routing(tv_core):
    """tv_core: [R] int types -> (g1 [NCALLS, CAP] i16 with -1 pads,
    cnt [NCALLS] i32, overflow core-local row ids needing host fixup)."""
    g1 = np.full((NCALLS, CAP), -1, np.int16)
    cnt = np.zeros(NCALLS, np.int32)
    overflow = []
    for sub in range(2):
        lo, hi = sub * SUB, (R if sub == 1 else SUB)
        tvs = tv_core[lo:hi]
        order = np.argsort(tvs, kind="stable")
        counts = np.bincount(tvs, minlength=T)
        start = 0
        for t in range(T):
            c = int(counts[t])
            seg = order[start:start + c]
            start += c
            k = sub * T + t
            if c > CAP:
                overflow.extend((seg[CAP:] + lo).tolist())
                seg, c = seg[:CAP], CAP
            if c == 0:
                # hardware path needs >=1 valid token per call; sacrifice
                # local row 0 (scatter adds garbage there; host recomputes)
                g1[k, 0] = 0
                cnt[k] = 1
                overflow.append(lo)
            else:
                g1[k, :c] = seg.astype(np.int16)
                cnt[k] = c
    return g1, cnt, overflow


def _wrap16(flat):
    """flat int16 [NCALLS*CAP] -> [16, NCALLS*COLS] wrapped (token i at
    [i%16, i//16]); replication to the 8 Q7 core groups happens on device."""
    return flat.reshape(-1, 16).T


def _host_mlp(m, w1, b1, w2, b2, w3, b3):
    h = np.maximum(m @ w1 + b1, 0)
    h = np.maximum(h @ w2 + b2, 0)
    return h @ w3 + b3


def kernel(**inputs):
    x = np.ascontiguousarray(np.asarray(inputs["x"], dtype=np.float32))
    tv = np.asarray(inputs["type_vec"]).astype(np.int64)
    assert x.shape == (N, IN_C), x.shape
    ef = np.asarray(inputs["edge_feas"], dtype=np.float32)

    # per-type weights/biases from the tiny generator MLPs (host, f32)
    W = _host_mlp(ef, *[np.asarray(inputs[k], dtype=np.float32) for k in
                        ("wg_w1", "wg_b1", "wg_w2", "wg_b2", "wg_w3", "wg_b3")]
                  ).reshape(T, IN_C, OUT_C)
    B = _host_mlp(ef, *[np.asarray(inputs[k], dtype=np.float32) for k in
                        ("bg_w1", "bg_b1", "bg_w2", "bg_b2", "bg_w3", "bg_b3")])

    if "runner" not in _CACHE:
        _CACHE["runner"] = _make_runner()
    sharded, zeros_fn, in_names = _CACHE["runner"]

    zeros = zeros_fn()  # async on-device; overlaps with host prep below
    # enqueue the big x transfer before doing routing work on host
    mesh, spec = _CACHE["mesh_spec"]
    x_dev = jax.device_put(x.astype(BF16), NamedSharding(mesh, spec))

    g1_g = np.empty((N_CORES * 16, NCALLS * COLS), np.int16)
    cnt_g = np.empty((N_CORES, NCALLS), np.int32)
    overflows = []
    for c in range(N_CORES):
        g1, cnt, ovf = _routing(tv[c * R:(c + 1) * R])
        g1_g[c * 16:(c + 1) * 16] = _wrap16(g1.reshape(-1))
        cnt_g[c] = cnt
        overflows.append(ovf)

    w_g = np.broadcast_to(
        np.ascontiguousarray(W.transpose(1, 0, 2)).astype(BF16),
        (N_CORES, IN_C, T, OUT_C)).reshape(N_CORES * IN_C, T, OUT_C)
    b_g = np.broadcast_to(B.reshape(1, T * OUT_C).astype(BF16),
                          (N_CORES, T * OUT_C))

    glob = {"x_s": x_dev, "g1idx": g1_g, "cnt": cnt_g,
            "wt": np.ascontiguousarray(w_g), "bt": np.ascontiguousarray(b_g)}
    out_arr = sharded(*[glob[n] for n in in_names], zeros)[0]

    out = np.empty((N, OUT_C), dtype=np.float32)
    from concurrent.futures import ThreadPoolExecutor

    def fetch(s):
        lo = s.index[0].start or 0
        out[lo:lo + R] = np.asarray(s.data)  # bf16 -> f32 cast on assign

    with ThreadPoolExecutor(4) as ex:
        list(ex.map(fetch, out_arr.addressable_shards))

    if any(overflows):  # per-type capacity overflow: recompute those rows
        for c in range(N_CORES):
            for rr in set(overflows[c]):
                g = c * R + rr
                t = int(tv[g])
                out[g] = x[g] @ W[t] + B[t]
    return out
